# revision 66
# baseline (speedup 1.0000x reference)
"""Trainium2 Bass kernel for nn_AdaptiveGraphConvLayer (graph multi-head attention).

Reference computation:
    mask = dense additive edge mask from edge_index (symmetric + self loops)
    per head h: q,k,v projections of x; scores = q @ k.T / 16 + mask; softmax
    o_h = attn @ v_h; head_out_h = o_h @ Wo_h.T + bo_h
    out = concat_h(head_out) @ Wp.T + bp;  LayerNorm(out) * gamma + beta
    (N=4096 nodes, D=256, H=4 heads, E=131072 edges; ~80 GFLOP)

Device strategy (node-parallel, zero collectives) — "factored" form:
  - Core c owns query rows [c*512, (c+1)*512) for ALL 4 heads.
  - Score-path fold:  scores = (x Wq^T)(x Wk^T)^T = x @ M @ x^T with
        M_h = Wq_h^T Wk_h   (host-precomputed, fp8 with a 64x scale)
    so the per-head K-projection over all 4096 nodes disappears: the
    scores matmul contracts raw fp8 x^T (landed once by DMA) against a
    tiny per-head tq = M^T x_q^T  [256, 512].
  - V-path fold:  o = attn^T (x Wv') = (attn^T x) Wv'  with
        Wv'_h = Wv_h^T (Wp_h Wo_h)^T   (host-precomputed)
    Stage 1 contracts the masked-exp tiles against RAW x chunks
    (head-independent, in SBUF once, ones-columns appended on host for
    the softmax denominators).  Stage 2 is a tiny per-head
    [512,256] @ [256,256] through PE transposes of the normalized G.
    The per-head V-projection over 4096 nodes and its PSUM->SBUF
    copies disappear.
  - fp8 DoubleRow matmuls for tq + scores; bf16 for stage 1/2.
  - Edge mask: host reshards edge_index into per-core dense {0,1} bf16
    stripes in SBUF layout; ONE DVE multiply per chunk right after its
    exp (the factored V path removed the projection copies that used
    to crowd DVE, so the mask is timely there; fp8 additive -240 mask
    via identity DR matmuls measured SLOWER: +27us PE LDWEIGHTS).
  - Stage-1 matmuls run TWO pairs behind the exp/mask pipeline so a
    queue hiccup ahead of an exp/mask in an engine FIFO eats lookahead,
    not PE time.  (Three pairs measured worse; pair-batched exps over
    [P,1024] PSUM tiles measured worse: 2 PSUM pair slots serialize
    scores(p+2) behind exp(p).)
  - fp32 PSUM accumulate, fp32 softmax-normalize/LayerNorm.
  - Head h's tail chain (normalize G, transpose, stage 2, acc) and head
    h+1's tq are emitted interleaved into head h+1's attention loop.
  - HAM clock gate: warmup matmuls hold 8/8 through the input-DMA window.
  - ACT activation-table sets: no set holds Exp+Sqrt.  The preamble
    loads the Exp set; Sqrt's set is loaded by a dummy activation
    anchored on the final et tile so the 1.3us switch hides under the
    last stage-1 matmuls instead of stalling the LN tail.
  - Tail: fused Square+accum_out variance, Sqrt table preloaded, affine
    LN ops elided when gamma/beta/bias are trivial for the given inputs.
"""

import numpy as np

N_FULL = 4096
D = 256
H = 4
N_CORES = 8
EPS = 1e-5
P = 128  # partitions

WARMUP = 5
SCALE_QK = 64.0  # host scale on M = Wq^T Wk so fp8 sees ~unit-rms values
# tq psum->fp8 copy scale: lands tq around unit rms (well clear of the
# fp8 denormal cliff); EXP_SCALE undoes SCALE_QK*TQ_BETA*sqrt(D)
TQ_BETA = 3.0 / 256.0
EXP_SCALE = 1.0 / 12.0


def _build(N, QW, mask_dt_name="bfloat16", mode="bf16",
           triv_bias=False, triv_gamma=False, triv_beta=False):
    """Build + compile the SPMD Bass graph (identical on all cores)."""
    import concourse.bacc as bacc
    import concourse.tile as tile
    import concourse.bass as bass
    from concourse import mybir

    f32 = mybir.dt.float32
    f8 = mybir.dt.float8e4
    mask_dt = getattr(mybir.dt, mask_dt_name)
    cdt = mybir.dt.bfloat16
    DR = mybir.MatmulPerfMode.DoubleRow
    Exp = mybir.ActivationFunctionType.Exp
    Sqrt = mybir.ActivationFunctionType.Sqrt
    AX = mybir.AxisListType.X
    MUL = mybir.AluOpType.mult
    KV = N // P            # kv chunks of 128
    NP2 = KV // 2          # kv chunk pairs
    QS = QW // P           # q slices of 128 within this core's window
    D1 = D + 2             # x + ones columns (even width)

    nc = bacc.Bacc("TRN2", target_bir_lowering=False, debug=False,
                   num_devices=N_CORES)

    xT8_d = nc.dram_tensor("xT8", [D, N], f8, kind="ExternalInput").ap()
    xq8_d = nc.dram_tensor("xq8", [D, QW], f8, kind="ExternalInput").ap()
    wqk8_d = nc.dram_tensor("wqk8", [H, D, D], f8, kind="ExternalInput").ap()
    xE_d = nc.dram_tensor("xE", [P, (N // P) * D1], cdt,
                          kind="ExternalInput").ap()
    wv_d = nc.dram_tensor("wv", [H, D, D], cdt, kind="ExternalInput").ap()
    idn_d = nc.dram_tensor("idn", [P, P], f32, kind="ExternalInput").ap()
    gam_d = nc.dram_tensor("gamma_b", [P, D], f32, kind="ExternalInput").ap()
    bet_d = nc.dram_tensor("beta_b", [P, D], f32, kind="ExternalInput").ap()
    bia_d = nc.dram_tensor("bias_b", [P, D], f32, kind="ExternalInput").ap()
    mal_d = nc.dram_tensor("mall", [P, (N // P) * QW], mask_dt,
                           kind="ExternalInput").ap()
    out_d = nc.dram_tensor("out", [QW, D], cdt, kind="ExternalOutput").ap()

    with tile.TileContext(nc) as tc:
        with (
            tc.tile_pool(name="const", bufs=1) as cp,
            tc.tile_pool(name="tqp", bufs=2) as tqp,
            tc.tile_pool(name="maskp", bufs=1) as mp,
            tc.tile_pool(name="work", bufs=8) as wp,
            tc.tile_pool(name="accs", bufs=1) as ac,
            tc.tile_pool(name="ln", bufs=6) as lp,
            tc.tile_pool(name="psA", bufs=4, space="PSUM") as psA,
            tc.tile_pool(name="psG", bufs=1, space="PSUM") as psG,
        ):
            # ---------- PE warmup: dummy matmuls on uninitialized SBUF so
            # the HAM clock-gate reaches K=8/8 while input DMAs stream in.
            wu = cp.tile([P, 640], cdt, tag="wu")
            nc.vector.memset(wu[:], 0.125)
            wups = psA.tile([P, 512], f32, tag="ps", name="wups")
            for w in range(WARMUP):
                nc.tensor.matmul(wups[:], lhsT=wu[:, :P], rhs=wu[:, P:P + 512],
                                 start=True, stop=True)

            # ---------- load inputs into SBUF ----------
            # DMA queue is FIFO: land the tq inputs first so the first real
            # matmuls start as early as possible.
            xq8 = cp.tile([P, 2 * QW], f8, tag="xq8")
            nc.sync.dma_start(out=xq8[:].rearrange("p (i q) -> p i q", q=QW),
                              in_=xq8_d[:].rearrange("(i p) q -> p i q", p=P))
            wqk8 = cp.tile([P, H * 2 * D], f8, tag="wqk8")
            nc.sync.dma_start(
                out=wqk8[:].rearrange("p (h i d) -> p h i d", h=H, i=2),
                in_=wqk8_d[:].rearrange("h (i p) d -> p h i d", p=P))
            # xT8 / {0,1} mask / xE stripes land interleaved by quarter:
            # head-0 chunk c waits only for its own quarter, so quarter 0 of
            # all three goes before quarter 1 of any
            xT8 = cp.tile([P, 2 * N], f8, tag="xT8")
            Mall = mp.tile([P, KV * QW], mask_dt, tag="mask")
            xE = cp.tile([P, KV * D1], cdt, tag="xE")
            MQ = KV // 4
            for q4 in range(4):
                w = N // 4
                nc.sync.dma_start(
                    out=xT8[:].rearrange("p (i n) -> p i n", n=N)
                        [:, :, q4 * w:(q4 + 1) * w],
                    in_=xT8_d[:].rearrange("(i p) n -> p i n", p=P)
                        [:, :, q4 * w:(q4 + 1) * w])
                nc.sync.dma_start(
                    out=Mall[:, q4 * MQ * QW:(q4 + 1) * MQ * QW],
                    in_=mal_d[:, q4 * MQ * QW:(q4 + 1) * MQ * QW])
                nc.sync.dma_start(
                    out=xE[:, q4 * MQ * D1:(q4 + 1) * MQ * D1],
                    in_=xE_d[:, q4 * MQ * D1:(q4 + 1) * MQ * D1])
            wv = cp.tile([P, H * 2 * D], cdt, tag="wv")
            nc.sync.dma_start(
                out=wv[:].rearrange("p (h i d) -> p h i d", h=H, i=2),
                in_=wv_d[:].rearrange("h (i p) d -> p h i d", p=P))
            idn = cp.tile([P, P], f32, tag="idn")
            nc.sync.dma_start(out=idn[:], in_=idn_d[:])
            gam = cp.tile([P, D], f32, tag="gam")
            bet = cp.tile([P, D], f32, tag="bet")
            bia = cp.tile([P, D], f32, tag="bia")
            if not triv_gamma:
                nc.sync.dma_start(out=gam[:], in_=gam_d[:])
            if not triv_beta:
                nc.sync.dma_start(out=bet[:], in_=bet_d[:])
            if not triv_bias:
                nc.sync.dma_start(out=bia[:], in_=bia_d[:])
            epsc = cp.tile([P, 1], f32, tag="epsc")
            nc.vector.memset(epsc[:], EPS)
            eps2 = cp.tile([P, 1], f32, tag="eps2")
            nc.vector.memset(eps2[:], float(D) * float(D) * EPS)
            # preload the Exp table set (covers Copy/Square too); Sqrt's set
            # is loaded late, anchored after the final exp
            sqwarm = cp.tile([P, 1], f32, tag="sqwarm")
            nc.scalar.activation(sqwarm[:], epsc[:], Exp, bias=epsc[:])

            xq8_r = xq8[:].rearrange("p (i q) -> p i q", i=2)
            xT8_r = xT8[:].rearrange("p (i n) -> p i n", i=2)
            wqk8_r = wqk8[:].rearrange("p (h i d) -> p h i d", h=H, i=2)
            xE_r = xE[:].rearrange("p (c e) -> p c e", e=D1)

            acc = [ac.tile([P, D], f32, tag=f"acc{s}", name=f"acc{s}")
                   for s in range(QS)]

            cpy = [0]

            def copy_eng():
                # 1:2 ACT:DVE (ACT carries the exp pipeline)
                cpy[0] += 1
                return nc.scalar if cpy[0] % 3 == 0 else nc.vector

            def emit_copy(dst, src):
                e = copy_eng()
                if e is nc.scalar:
                    e.copy(dst, src)
                else:
                    e.tensor_copy(dst, src)

            def mk_tq(h):
                """tq_h = (M_h^T x_q^T) as 2 fp8 DR planes [P, 2, QW]."""
                tq = tqp.tile([P, 2 * QW], f8, tag="tq", name=f"tq{h}")

                def emit():
                    Copy = mybir.ActivationFunctionType.Copy
                    for j in range(2):
                        ps = psA.tile([P, 512], f32, tag="ps",
                                      name=f"tq{h}ps{j}")
                        nc.tensor.matmul(
                            ps[:, :QW],
                            lhsT=wqk8_r[:, h, :, j * P:(j + 1) * P],
                            rhs=xq8_r, start=True, stop=True, perf_mode=DR)
                        # scaled copy: lands tq so scores psum is 12*s and
                        # the fp8 -240 mask add zeroes via exp
                        if j == 0:
                            nc.scalar.activation(tq[:, :QW], ps[:, :QW],
                                                 Copy, scale=TQ_BETA)
                        else:
                            nc.vector.tensor_scalar(
                                out=tq[:, QW:], in0=ps[:, :QW],
                                scalar1=TQ_BETA, scalar2=None, op0=MUL)
                return tq, emit

            inv_d = 1.0 / D
            Square = mybir.ActivationFunctionType.Square

            def ln_slice(s):
                """bias + LayerNorm + store for one q slice."""
                t = acc[s]
                if not triv_bias:
                    nc.vector.tensor_add(t[:], t[:], bia[:])
                # one-pass LN stats: sum on DVE and sum-of-squares on ACT in
                # parallel on t; then D^2*var = D*sumsq - sum^2 via [P,1]
                # ops.  LN is scale-invariant: y = (D*t - sum)/sqrt(D^2*var
                # + D^2*eps).
                musum = lp.tile([P, 1], f32, tag="musum")
                nc.vector.reduce_sum(musum[:], t[:], axis=AX)
                sq = lp.tile([P, D], f32, tag="sq")
                vs = lp.tile([P, 1], f32, tag="vs")
                nc.scalar.activation(sq[:], t[:], Square, accum_out=vs[:])
                xc = lp.tile([P, D], f32, tag="xc")
                nc.vector.tensor_scalar(out=xc[:], in0=t[:], scalar1=float(D),
                                        scalar2=musum[:],
                                        op0=MUL, op1=mybir.AluOpType.subtract)
                t1 = lp.tile([P, 1], f32, tag="t1")
                nc.vector.tensor_mul(t1[:], musum[:], musum[:])
                t2 = lp.tile([P, 1], f32, tag="t2")
                nc.vector.tensor_scalar(out=t2[:], in0=vs[:],
                                        scalar1=float(D), scalar2=t1[:],
                                        op0=MUL, op1=mybir.AluOpType.subtract)
                sd = lp.tile([P, 1], f32, tag="sd")
                nc.scalar.activation(sd[:], t2[:], Sqrt, bias=eps2[:])
                rs = lp.tile([P, 1], f32, tag="rs")
                nc.vector.reciprocal(rs[:], sd[:])
                og = lp.tile([P, D], cdt, tag="og")
                if triv_gamma:
                    nc.vector.tensor_scalar_mul(og[:], xc[:], rs[:])
                else:
                    nc.vector.scalar_tensor_tensor(og[:], in0=xc[:],
                                                   scalar=rs[:], in1=gam[:],
                                                   op0=MUL, op1=MUL)
                if triv_beta:
                    nc.sync.dma_start(out=out_d[s * P:(s + 1) * P, :],
                                      in_=og[:])
                else:
                    oo = lp.tile([P, D], cdt, tag="oo")
                    nc.vector.tensor_add(oo[:], og[:], bet[:])
                    nc.sync.dma_start(out=out_d[s * P:(s + 1) * P, :],
                                      in_=oo[:])

            # head 0's tq runs in the prologue
            tq0, emit0 = mk_tq(0)
            emit0()
            cur_tq = tq0

            pend = []  # thunks spread into the current head's kv loop

            for h in range(H):
                tq_r = cur_tq[:].rearrange("p (i q) -> p i q", i=2)
                G = [psG.tile([P, D1], f32, tag=f"G{s}", name=f"G{s}_{h}")
                     for s in range(QS)]

                def stage1(c, et2, G=G):
                    u = c % 2
                    for s in range(QS):
                        nc.tensor.matmul(
                            G[s][:],
                            lhsT=et2[:, u * QW + s * P:u * QW + (s + 1) * P],
                            rhs=xE_r[:, c], start=(c == 0), stop=(c == KV - 1))

                # next head's tq: tiny (2 DR matmuls + 2 copies); MUST be
                # emitted inside THIS head's loop so the next head's scores
                # reads are ordered after its writes
                if h + 1 < H:
                    nxt_tq, emit_tq = mk_tq(h + 1)
                else:
                    nxt_tq, emit_tq = None, None

                npend = len(pend)
                pairs = []
                et2 = None
                for c in range(KV):
                    u = c % 2
                    sc = psA.tile([P, 512], f32, tag="ps")
                    nc.tensor.matmul(sc[:, :QW],
                                     lhsT=xT8_r[:, :, c * P:c * P + P],
                                     rhs=tq_r, start=True, stop=True,
                                     perf_mode=DR)
                    if u == 0:
                        et2 = wp.tile([P, 2 * QW], cdt, tag="et")
                    nc.scalar.activation(et2[:, u * QW:(u + 1) * QW],
                                         sc[:, :QW], Exp, scale=EXP_SCALE)
                    # per-chunk {0,1} mask multiply on DVE right after the
                    # exp: the even half unblocks its stage-1 a chunk sooner
                    nc.vector.tensor_mul(et2[:, u * QW:(u + 1) * QW],
                                         et2[:, u * QW:(u + 1) * QW],
                                         Mall[:, c * QW:(c + 1) * QW])
                    if u == 1:
                        pairs.append(et2)
                    if c >= 4:
                        stage1(c - 4, pairs[(c - 4) // 2])
                    if c == 6 and emit_tq is not None:
                        emit_tq()
                    if c >= 2 and npend:
                        want = ((c - 1) * npend) // (KV - 2)
                        while npend - len(pend) < want and pend:
                            pend.pop(0)()

                def slice_A(s, G=G):
                    rec = lp.tile([P, 1], f32, tag="rec")
                    nc.vector.reciprocal(rec[:], G[s][:, D:D + 1])
                    gn = lp.tile([P, D], f32, tag="gn", name=f"gn{s}_{h}")
                    nc.vector.tensor_scalar_mul(gn[:], G[s][:, 0:D], rec[:])
                    return gn

                def slice_BC(s, gn, h=h):
                    tp = psA.tile([P, 512], f32, tag="ps", name=f"tp{s}_{h}")
                    nc.tensor.transpose(tp[:, 0:P], gn[:, 0:P], idn[:])
                    nc.tensor.transpose(tp[:, P:2 * P], gn[:, P:2 * P],
                                        idn[:])
                    gt = lp.tile([P, 2 * P], cdt, tag="gt", name=f"gt{s}_{h}")
                    emit_copy(gt[:], tp[:, 0:2 * P])
                    s2 = psA.tile([P, 512], f32, tag="ps", name=f"s2{s}_{h}")
                    for i in range(2):
                        nc.tensor.matmul(
                            s2[:, :D], lhsT=gt[:, i * P:(i + 1) * P],
                            rhs=wv[:, (h * 2 + i) * D:(h * 2 + i + 1) * D],
                            start=(i == 0), stop=(i == 1))
                    if h == 0:
                        nc.vector.tensor_copy(acc[s][:], s2[:, :D])
                    else:
                        nc.vector.tensor_add(acc[s][:], s2[:, :D], acc[s][:])

                if h < H - 1:
                    for c in range(KV - 4, KV):
                        stage1(c, pairs[c // 2])
                    while pend:
                        pend.pop(0)()
                    # ---- head tail chain ----
                    # A: normalize G by its denominator, emitted NOW (the
                    # next head's stage-1 start=True on the same PSUM tags
                    # must be emitted after these reads); B/C spread into the
                    # next head's attention loop
                    gns = [slice_A(s) for s in range(QS)]

                    def mk_bc(s, gns=gns):
                        return lambda: slice_BC(s, gns[s])

                    for s in range(QS):
                        pend.append(mk_bc(s))
                    cur_tq = nxt_tq
                else:
                    # last head: per-slice pipelined drain -> normalize ->
                    # stage2 -> LayerNorm, with dummy matmuls holding the
                    # HAM clock gate at 8/8 through the tail
                    nc.scalar.activation(sqwarm[:], pairs[-1][:, 0:1], Sqrt,
                                         bias=epsc[:])
                    while pend:
                        pend.pop(0)()

                    def drain_s(s):
                        for c in range(KV - 4, KV):
                            u = c % 2
                            et2d = pairs[c // 2]
                            nc.tensor.matmul(
                                G[s][:],
                                lhsT=et2d[:, u * QW + s * P:
                                          u * QW + (s + 1) * P],
                                rhs=xE_r[:, c], start=False,
                                stop=(c == KV - 1))

                    drain_s(0)
                    gns3 = [None] * QS
                    for s in range(QS):
                        gns3[s] = slice_A(s)
                        if s + 1 < QS:
                            # next slice's drain fills the PE while DVE runs
                            # this slice's normalize
                            drain_s(s + 1)
                        slice_BC(s, gns3[s])
                        for w in range(3):
                            wut = psA.tile([P, 512], f32, tag="ps",
                                           name=f"wt{s}_{w}")
                            nc.tensor.matmul(wut[:], lhsT=wu[:, :P],
                                             rhs=wu[:, P:P + 512],
                                             start=True, stop=True)
                        ln_slice(s)

    nc.compile()
    return nc


def _prep_host(inputs, N, QW):
    """Host-side input resharding: transposes, folded weights, mask stripes."""
    import ml_dtypes
    x = np.ascontiguousarray(np.asarray(inputs["x"], dtype=np.float32))
    ei = np.asarray(inputs["edge_index"]).astype(np.int64)
    Wq = np.asarray(inputs["Wq"], dtype=np.float64)
    Wk = np.asarray(inputs["Wk"], dtype=np.float64)
    Wv = np.asarray(inputs["Wv"], dtype=np.float64)
    Wo = np.asarray(inputs["Wo"], dtype=np.float64)
    Wp = np.asarray(inputs["Wp"], dtype=np.float64)
    bq = np.asarray(inputs["bq"], dtype=np.float64)
    bk = np.asarray(inputs["bk"], dtype=np.float64)
    bv = np.asarray(inputs["bv"], dtype=np.float64)
    bo = np.asarray(inputs["bo"], dtype=np.float64)
    bp = np.asarray(inputs["bp"], dtype=np.float64)
    gamma = np.asarray(inputs["gamma"], dtype=np.float32)
    beta = np.asarray(inputs["beta"], dtype=np.float32)

    assert not bq.any() and not bk.any(), \
        "nonzero q/k biases not wired in the device graph"

    xT = np.ascontiguousarray(x.T)                       # [D, N]
    # folded score weight M_h = Wq_h^T Wk_h (x M x^T == q k^T)
    m_l = []
    for hh in range(H):
        m_l.append((Wq[hh].T @ Wk[hh]) * SCALE_QK)
    wqk_h = np.ascontiguousarray(np.stack(m_l).astype(np.float32))
    # folded v' weight and total bias
    wv_l, bias_tot = [], bp.copy()
    for hh in range(H):
        Wp_h = Wp[:, hh * D:(hh + 1) * D]                # [f, e']
        Gm = Wo[hh].T @ Wp_h.T                           # [e, f]
        wv_l.append(Wv[hh].T @ Gm)                       # [d, f]
        bias_tot = bias_tot + bo[hh] @ Wp_h.T + bv[hh] @ Gm
    wv_h = np.ascontiguousarray(np.stack(wv_l).astype(np.float32))

    gam_b = np.ascontiguousarray(np.broadcast_to(gamma, (P, D)).astype(np.float32))
    bet_b = np.ascontiguousarray(np.broadcast_to(beta, (P, D)).astype(np.float32))
    bia_b = np.ascontiguousarray(
        np.broadcast_to(bias_tot.astype(np.float32), (P, D)))

    KV = N // P
    D1 = D + 2
    # raw x chunks + ones columns, pre-arranged to the SBUF layout
    xE = np.ones((N, D1), dtype=np.float32)
    xE[:, :D] = x
    xE_b = np.ascontiguousarray(
        xE.reshape(KV, P, D1).transpose(1, 0, 2).reshape(P, KV * D1)
        .astype(ml_dtypes.bfloat16))

    # {0,1} mask stripes per core, pre-arranged to the SBUF layout
    # mall[p, c*QW + q] = adjacency[c*P + p, q0 + q]  (kv-major, symm+diag)
    adj = np.zeros((N, N), dtype=np.uint8)
    r, c = ei[0], ei[1]
    adj[r, c] = 1
    adj[c, r] = 1
    adj[np.arange(N), np.arange(N)] = 1
    malls = []
    for core in range(N_CORES):
        q0 = core * QW
        stripe = adj[:, q0:q0 + QW]                      # [N(kv), QW]
        m = stripe.reshape(KV, P, QW).transpose(1, 0, 2).reshape(P, KV * QW)
        malls.append(np.ascontiguousarray(m.astype(ml_dtypes.bfloat16)))
    return xT, wqk_h, wv_h, xE_b, gam_b, bet_b, bia_b, malls


_BUILD_CACHE = {}


def _run(inputs, trace=False, mask_dt_name="bfloat16", mode="bf16",
         tmpdir=None):
    from concourse.bass_utils import run_bass_kernel_spmd
    from concourse.bass_interp import get_hw_module
    import ml_dtypes

    N = int(np.asarray(inputs["x"]).shape[0])
    QW = N // N_CORES
    (xT, wqk_h, wv_h, xE_b, gam_b, bet_b, bia_b, malls) = \
        _prep_host(inputs, N, QW)

    bdt = ml_dtypes.bfloat16
    f8dt = ml_dtypes.float8_e4m3
    xT_8 = xT.astype(f8dt)
    wqk_8 = wqk_h.astype(f8dt)
    wv_b = wv_h.astype(bdt)
    idn_b = np.eye(P, dtype=np.float32)

    gamma = np.asarray(inputs["gamma"], np.float64)
    beta = np.asarray(inputs["beta"], np.float64)
    key = (N, QW, mask_dt_name, mode, not np.any(bia_b),
           bool((gamma == 1).all()), not beta.any())
    nc = _BUILD_CACHE.get(key)
    if nc is None:
        nc = _build(N, QW, mask_dt_name=mask_dt_name, mode=mode,
                    triv_bias=key[4], triv_gamma=key[5], triv_beta=key[6])
        _BUILD_CACHE[key] = nc
    old = nc.m
    nc.m = get_hw_module(nc.m)
    try:
        in_maps = []
        for core in range(N_CORES):
            q0 = core * QW
            in_maps.append({
                "xT8": xT_8,
                "xq8": np.ascontiguousarray(xT_8[:, q0:q0 + QW]),
                "wqk8": wqk_8, "wv": wv_b, "xE": xE_b, "idn": idn_b,
                "gamma_b": gam_b, "beta_b": bet_b, "bias_b": bia_b,
                "mall": malls[core],
            })
        res = run_bass_kernel_spmd(nc, in_maps, core_ids=list(range(N_CORES)),
                                   trace=trace, tmpdir=tmpdir)
    finally:
        nc.m = old
    out = np.concatenate([res.results[i]["out"] for i in range(N_CORES)], axis=0)
    return out.astype(np.float32), res


def kernel(**inputs) -> np.ndarray:
    out, _ = _run(inputs)
    return out


# revision 67
# speedup vs baseline: 1.0302x; 1.0302x over previous
"""Trainium2 Bass kernel for nn_AdaptiveGraphConvLayer (graph multi-head attention).

Reference computation:
    mask = dense additive edge mask from edge_index (symmetric + self loops)
    per head h: q,k,v projections of x; scores = q @ k.T / 16 + mask; softmax
    o_h = attn @ v_h; head_out_h = o_h @ Wo_h.T + bo_h
    out = concat_h(head_out) @ Wp.T + bp;  LayerNorm(out) * gamma + beta
    (N=4096 nodes, D=256, H=4 heads, E=131072 edges; ~80 GFLOP)

Device strategy (node-parallel, zero collectives) — "factored" form:
  - Core c owns query rows [c*512, (c+1)*512) for ALL 4 heads.
  - Score-path fold:  scores = (x Wq^T)(x Wk^T)^T = x @ M @ x^T with
        M_h = Wq_h^T Wk_h   (host-precomputed, fp8 with a 64x scale)
    so the per-head K-projection over all 4096 nodes disappears: the
    scores matmul contracts raw fp8 x^T (landed once by DMA) against a
    tiny per-head tq = M^T x_q^T  [256, 512].
  - V-path fold:  o = attn^T (x Wv') = (attn^T x) Wv'  with
        Wv'_h = Wv_h^T (Wp_h Wo_h)^T   (host-precomputed)
    Stage 1 contracts the masked-exp tiles against RAW x chunks
    (head-independent, in SBUF once, ones-columns appended on host for
    the softmax denominators).  Stage 2 is a tiny per-head
    [512,256] @ [256,256] through PE transposes of the normalized G.
    The per-head V-projection over 4096 nodes and its PSUM->SBUF
    copies disappear.
  - fp8 DoubleRow matmuls for tq + scores; bf16 for stage 1/2.
  - Edge mask: host reshards edge_index into per-core dense {0,1} bf16
    stripes in SBUF layout; ONE DVE multiply per chunk right after its
    exp (the factored V path removed the projection copies that used
    to crowd DVE, so the mask is timely there; fp8 additive -240 mask
    via identity DR matmuls measured SLOWER: +27us PE LDWEIGHTS).
  - Stage-1 matmuls run TWO pairs behind the exp/mask pipeline so a
    queue hiccup ahead of an exp/mask in an engine FIFO eats lookahead,
    not PE time.  (Three pairs measured worse; pair-batched exps over
    [P,1024] PSUM tiles measured worse: 2 PSUM pair slots serialize
    scores(p+2) behind exp(p).)
  - fp32 PSUM accumulate, fp32 softmax-normalize/LayerNorm.
  - Head h's tail chain (normalize G, transpose, stage 2, acc) and head
    h+1's tq are emitted interleaved into head h+1's attention loop.
  - HAM clock gate: warmup matmuls hold 8/8 through the input-DMA window.
  - ACT activation-table sets: no set holds Exp+Sqrt.  The preamble
    loads the Exp set; Sqrt's set is loaded by a dummy activation
    anchored on the final et tile so the 1.3us switch hides under the
    last stage-1 matmuls instead of stalling the LN tail.
  - Tail: fused Square+accum_out variance, Sqrt table preloaded, affine
    LN ops elided when gamma/beta/bias are trivial for the given inputs.
"""

import numpy as np

N_FULL = 4096
D = 256
H = 4
N_CORES = 8
EPS = 1e-5
P = 128  # partitions

WARMUP = 5
SCALE_QK = 64.0  # host scale on M = Wq^T Wk so fp8 sees ~unit-rms values
# tq psum->fp8 copy scale: lands tq around unit rms (well clear of the
# fp8 denormal cliff); EXP_SCALE undoes SCALE_QK*TQ_BETA*sqrt(D)
TQ_BETA = 3.0 / 256.0
EXP_SCALE = 1.0 / 12.0


def _build(N, QW, mask_dt_name="bfloat16", mode="bf16",
           triv_bias=False, triv_gamma=False, triv_beta=False):
    """Build + compile the SPMD Bass graph (identical on all cores)."""
    import concourse.bacc as bacc
    import concourse.tile as tile
    import concourse.bass as bass
    from concourse import mybir

    f32 = mybir.dt.float32
    f8 = mybir.dt.float8e4
    mask_dt = getattr(mybir.dt, mask_dt_name)
    cdt = mybir.dt.bfloat16
    DR = mybir.MatmulPerfMode.DoubleRow
    Exp = mybir.ActivationFunctionType.Exp
    Sqrt = mybir.ActivationFunctionType.Sqrt
    AX = mybir.AxisListType.X
    MUL = mybir.AluOpType.mult
    KV = N // P            # kv chunks of 128
    NP2 = KV // 2          # kv chunk pairs
    QS = QW // P           # q slices of 128 within this core's window
    D1 = D + 2             # x + ones columns (even width)

    nc = bacc.Bacc("TRN2", target_bir_lowering=False, debug=False,
                   num_devices=N_CORES)

    xT8_d = nc.dram_tensor("xT8", [D, N], f8, kind="ExternalInput").ap()
    xq8_d = nc.dram_tensor("xq8", [D, QW], f8, kind="ExternalInput").ap()
    wqk8_d = nc.dram_tensor("wqk8", [H, D, D], f8, kind="ExternalInput").ap()
    xE_d = nc.dram_tensor("xE", [P, (N // P) * D1], cdt,
                          kind="ExternalInput").ap()
    wv_d = nc.dram_tensor("wv", [H, D, D], cdt, kind="ExternalInput").ap()
    idn_d = nc.dram_tensor("idn", [P, P], cdt, kind="ExternalInput").ap()
    gam_d = nc.dram_tensor("gamma_b", [P, D], f32, kind="ExternalInput").ap()
    bet_d = nc.dram_tensor("beta_b", [P, D], f32, kind="ExternalInput").ap()
    bia_d = nc.dram_tensor("bias_b", [P, D], f32, kind="ExternalInput").ap()
    mal_d = nc.dram_tensor("mall", [P, (N // P) * QW], mask_dt,
                           kind="ExternalInput").ap()
    out_d = nc.dram_tensor("out", [QW, D], cdt, kind="ExternalOutput").ap()

    with tile.TileContext(nc) as tc:
        with (
            tc.tile_pool(name="const", bufs=1) as cp,
            tc.tile_pool(name="tqp", bufs=2) as tqp,
            tc.tile_pool(name="maskp", bufs=1) as mp,
            tc.tile_pool(name="work", bufs=8) as wp,
            tc.tile_pool(name="accs", bufs=1) as ac,
            tc.tile_pool(name="ln", bufs=6) as lp,
            tc.tile_pool(name="psA", bufs=4, space="PSUM") as psA,
            tc.tile_pool(name="psG", bufs=1, space="PSUM") as psG,
        ):
            # ---------- PE warmup: dummy matmuls on uninitialized SBUF so
            # the HAM clock-gate reaches K=8/8 while input DMAs stream in.
            wu = cp.tile([P, 640], cdt, tag="wu")
            nc.vector.memset(wu[:], 0.125)
            wups = psA.tile([P, 512], f32, tag="ps", name="wups")
            for w in range(WARMUP):
                nc.tensor.matmul(wups[:], lhsT=wu[:, :P], rhs=wu[:, P:P + 512],
                                 start=True, stop=True)

            # ---------- load inputs into SBUF ----------
            # DMA queue is FIFO: land the tq inputs first so the first real
            # matmuls start as early as possible.
            xq8 = cp.tile([P, 2 * QW], f8, tag="xq8")
            nc.sync.dma_start(out=xq8[:].rearrange("p (i q) -> p i q", q=QW),
                              in_=xq8_d[:].rearrange("(i p) q -> p i q", p=P))
            wqk8 = cp.tile([P, H * 2 * D], f8, tag="wqk8")
            nc.sync.dma_start(
                out=wqk8[:].rearrange("p (h i d) -> p h i d", h=H, i=2),
                in_=wqk8_d[:].rearrange("h (i p) d -> p h i d", p=P))
            # xT8 / {0,1} mask / xE stripes land interleaved by quarter:
            # head-0 chunk c waits only for its own quarter, so quarter 0 of
            # all three goes before quarter 1 of any
            xT8 = cp.tile([P, 2 * N], f8, tag="xT8")
            Mall = mp.tile([P, KV * QW], mask_dt, tag="mask")
            xE = cp.tile([P, KV * D1], cdt, tag="xE")
            MQ = KV // 4
            for q4 in range(4):
                w = N // 4
                nc.sync.dma_start(
                    out=xT8[:].rearrange("p (i n) -> p i n", n=N)
                        [:, :, q4 * w:(q4 + 1) * w],
                    in_=xT8_d[:].rearrange("(i p) n -> p i n", p=P)
                        [:, :, q4 * w:(q4 + 1) * w])
                nc.sync.dma_start(
                    out=Mall[:, q4 * MQ * QW:(q4 + 1) * MQ * QW],
                    in_=mal_d[:, q4 * MQ * QW:(q4 + 1) * MQ * QW])
                nc.sync.dma_start(
                    out=xE[:, q4 * MQ * D1:(q4 + 1) * MQ * D1],
                    in_=xE_d[:, q4 * MQ * D1:(q4 + 1) * MQ * D1])
            wv = cp.tile([P, H * 2 * D], cdt, tag="wv")
            nc.sync.dma_start(
                out=wv[:].rearrange("p (h i d) -> p h i d", h=H, i=2),
                in_=wv_d[:].rearrange("h (i p) d -> p h i d", p=P))
            idn = cp.tile([P, P], cdt, tag="idn")
            nc.sync.dma_start(out=idn[:], in_=idn_d[:])
            gam = cp.tile([P, D], f32, tag="gam")
            bet = cp.tile([P, D], f32, tag="bet")
            bia = cp.tile([P, D], f32, tag="bia")
            if not triv_gamma:
                nc.sync.dma_start(out=gam[:], in_=gam_d[:])
            if not triv_beta:
                nc.sync.dma_start(out=bet[:], in_=bet_d[:])
            if not triv_bias:
                nc.sync.dma_start(out=bia[:], in_=bia_d[:])
            epsc = cp.tile([P, 1], f32, tag="epsc")
            nc.vector.memset(epsc[:], EPS)
            eps2 = cp.tile([P, 1], f32, tag="eps2")
            nc.vector.memset(eps2[:], float(D) * float(D) * EPS)
            # preload the Exp table set (covers Copy/Square too); Sqrt's set
            # is loaded late, anchored after the final exp
            sqwarm = cp.tile([P, 1], f32, tag="sqwarm")
            nc.scalar.activation(sqwarm[:], epsc[:], Exp, bias=epsc[:])

            xq8_r = xq8[:].rearrange("p (i q) -> p i q", i=2)
            xT8_r = xT8[:].rearrange("p (i n) -> p i n", i=2)
            wqk8_r = wqk8[:].rearrange("p (h i d) -> p h i d", h=H, i=2)
            xE_r = xE[:].rearrange("p (c e) -> p c e", e=D1)

            acc = [ac.tile([P, D], f32, tag=f"acc{s}", name=f"acc{s}")
                   for s in range(QS)]

            cpy = [0]

            def copy_eng():
                # 1:2 ACT:DVE (ACT carries the exp pipeline)
                cpy[0] += 1
                return nc.scalar if cpy[0] % 3 == 0 else nc.vector

            def emit_copy(dst, src):
                e = copy_eng()
                if e is nc.scalar:
                    e.copy(dst, src)
                else:
                    e.tensor_copy(dst, src)

            def mk_tq(h):
                """tq_h = (M_h^T x_q^T) as 2 fp8 DR planes [P, 2, QW]."""
                tq = tqp.tile([P, 2 * QW], f8, tag="tq", name=f"tq{h}")

                def emit():
                    Copy = mybir.ActivationFunctionType.Copy
                    for j in range(2):
                        ps = psA.tile([P, 512], f32, tag="ps",
                                      name=f"tq{h}ps{j}")
                        nc.tensor.matmul(
                            ps[:, :QW],
                            lhsT=wqk8_r[:, h, :, j * P:(j + 1) * P],
                            rhs=xq8_r, start=True, stop=True, perf_mode=DR)
                        # scaled copy: lands tq so scores psum is 12*s and
                        # the fp8 -240 mask add zeroes via exp
                        if j == 0:
                            nc.scalar.activation(tq[:, :QW], ps[:, :QW],
                                                 Copy, scale=TQ_BETA)
                        else:
                            nc.vector.tensor_scalar(
                                out=tq[:, QW:], in0=ps[:, :QW],
                                scalar1=TQ_BETA, scalar2=None, op0=MUL)
                return tq, emit

            inv_d = 1.0 / D
            Square = mybir.ActivationFunctionType.Square

            def ln_slice(s):
                """bias + LayerNorm + store for one q slice."""
                t = acc[s]
                if not triv_bias:
                    nc.vector.tensor_add(t[:], t[:], bia[:])
                # one-pass LN stats: sum on DVE and sum-of-squares on ACT in
                # parallel on t; then D^2*var = D*sumsq - sum^2 via [P,1]
                # ops.  LN is scale-invariant: y = (D*t - sum)/sqrt(D^2*var
                # + D^2*eps).
                musum = lp.tile([P, 1], f32, tag="musum")
                nc.vector.reduce_sum(musum[:], t[:], axis=AX)
                sq = lp.tile([P, D], f32, tag="sq")
                vs = lp.tile([P, 1], f32, tag="vs")
                nc.scalar.activation(sq[:], t[:], Square, accum_out=vs[:])
                xc = lp.tile([P, D], f32, tag="xc")
                nc.vector.tensor_scalar(out=xc[:], in0=t[:], scalar1=float(D),
                                        scalar2=musum[:],
                                        op0=MUL, op1=mybir.AluOpType.subtract)
                t1 = lp.tile([P, 1], f32, tag="t1")
                nc.vector.tensor_mul(t1[:], musum[:], musum[:])
                t2 = lp.tile([P, 1], f32, tag="t2")
                nc.vector.tensor_scalar(out=t2[:], in0=vs[:],
                                        scalar1=float(D), scalar2=t1[:],
                                        op0=MUL, op1=mybir.AluOpType.subtract)
                sd = lp.tile([P, 1], f32, tag="sd")
                nc.scalar.activation(sd[:], t2[:], Sqrt, bias=eps2[:])
                rs = lp.tile([P, 1], f32, tag="rs")
                nc.vector.reciprocal(rs[:], sd[:])
                og = lp.tile([P, D], cdt, tag="og")
                if triv_gamma:
                    nc.vector.tensor_scalar_mul(og[:], xc[:], rs[:])
                else:
                    nc.vector.scalar_tensor_tensor(og[:], in0=xc[:],
                                                   scalar=rs[:], in1=gam[:],
                                                   op0=MUL, op1=MUL)
                if triv_beta:
                    nc.sync.dma_start(out=out_d[s * P:(s + 1) * P, :],
                                      in_=og[:])
                else:
                    oo = lp.tile([P, D], cdt, tag="oo")
                    nc.vector.tensor_add(oo[:], og[:], bet[:])
                    nc.sync.dma_start(out=out_d[s * P:(s + 1) * P, :],
                                      in_=oo[:])

            # head 0's tq runs in the prologue
            tq0, emit0 = mk_tq(0)
            emit0()
            cur_tq = tq0

            pend = []  # thunks spread into the current head's kv loop

            for h in range(H):
                tq_r = cur_tq[:].rearrange("p (i q) -> p i q", i=2)
                G = [psG.tile([P, D1], f32, tag=f"G{s}", name=f"G{s}_{h}")
                     for s in range(QS)]

                def stage1(c, et2, G=G):
                    u = c % 2
                    for s in range(QS):
                        nc.tensor.matmul(
                            G[s][:],
                            lhsT=et2[:, u * QW + s * P:u * QW + (s + 1) * P],
                            rhs=xE_r[:, c], start=(c == 0), stop=(c == KV - 1))

                # next head's tq: tiny (2 DR matmuls + 2 copies); MUST be
                # emitted inside THIS head's loop so the next head's scores
                # reads are ordered after its writes
                if h + 1 < H:
                    nxt_tq, emit_tq = mk_tq(h + 1)
                else:
                    nxt_tq, emit_tq = None, None

                npend = len(pend)
                pairs = []
                et2 = None
                for c in range(KV):
                    u = c % 2
                    sc = psA.tile([P, 512], f32, tag="ps")
                    nc.tensor.matmul(sc[:, :QW],
                                     lhsT=xT8_r[:, :, c * P:c * P + P],
                                     rhs=tq_r, start=True, stop=True,
                                     perf_mode=DR)
                    if u == 0:
                        et2 = wp.tile([P, 2 * QW], cdt, tag="et")
                    nc.scalar.activation(et2[:, u * QW:(u + 1) * QW],
                                         sc[:, :QW], Exp, scale=EXP_SCALE)
                    # per-chunk {0,1} mask multiply on DVE right after the
                    # exp: the even half unblocks its stage-1 a chunk sooner
                    nc.vector.tensor_mul(et2[:, u * QW:(u + 1) * QW],
                                         et2[:, u * QW:(u + 1) * QW],
                                         Mall[:, c * QW:(c + 1) * QW])
                    if u == 1:
                        pairs.append(et2)
                    if c >= 4:
                        stage1(c - 4, pairs[(c - 4) // 2])
                    if c == 6 and emit_tq is not None:
                        emit_tq()
                    if c >= 2 and npend:
                        want = ((c - 1) * npend) // (KV - 2)
                        while npend - len(pend) < want and pend:
                            pend.pop(0)()

                def slice_A(s, G=G):
                    # reciprocal of the denominator column + UNNORMALIZED
                    # bf16 copy of G: the 1/denom folds into the acc update
                    # (keeps the PE transpose chain off the normalize)
                    rec = lp.tile([P, 1], f32, tag="rec")
                    nc.vector.reciprocal(rec[:], G[s][:, D:D + 1])
                    gn = lp.tile([P, D], cdt, tag="gn", name=f"gn{s}_{h}")
                    nc.vector.tensor_copy(gn[:], G[s][:, 0:D])
                    return (rec, gn)

                def slice_BC(s, recgn, h=h):
                    rec, gn = recgn
                    # bf16 transposes (psum bf16 tile = same 2KB footprint
                    # as the f32 scores tiles) + a 2x-mode gt copy
                    tp = psA.tile([P, 1024], cdt, tag="ps", name=f"tp{s}_{h}")
                    nc.tensor.transpose(tp[:, 0:P], gn[:, 0:P], idn[:])
                    nc.tensor.transpose(tp[:, P:2 * P], gn[:, P:2 * P],
                                        idn[:])
                    gt = lp.tile([P, 2 * P], cdt, tag="gt", name=f"gt{s}_{h}")
                    emit_copy(gt[:], tp[:, 0:2 * P])
                    s2 = psA.tile([P, 512], f32, tag="ps", name=f"s2{s}_{h}")
                    for i in range(2):
                        nc.tensor.matmul(
                            s2[:, :D], lhsT=gt[:, i * P:(i + 1) * P],
                            rhs=wv[:, (h * 2 + i) * D:(h * 2 + i + 1) * D],
                            start=(i == 0), stop=(i == 1))
                    if h == 0:
                        nc.vector.tensor_scalar_mul(acc[s][:], s2[:, :D],
                                                    rec[:])
                    else:
                        nc.vector.scalar_tensor_tensor(
                            acc[s][:], in0=s2[:, :D], scalar=rec[:],
                            in1=acc[s][:], op0=MUL,
                            op1=mybir.AluOpType.add)

                if h < H - 1:
                    for c in range(KV - 4, KV):
                        stage1(c, pairs[c // 2])
                    while pend:
                        pend.pop(0)()
                    # ---- head tail chain ----
                    # A: normalize G by its denominator, emitted NOW (the
                    # next head's stage-1 start=True on the same PSUM tags
                    # must be emitted after these reads); B/C spread into the
                    # next head's attention loop
                    gns = [slice_A(s) for s in range(QS)]

                    def mk_bc(s, gns=gns):
                        return lambda: slice_BC(s, gns[s])

                    for s in range(QS):
                        pend.append(mk_bc(s))
                    cur_tq = nxt_tq
                else:
                    # last head: per-slice pipelined drain -> normalize ->
                    # stage2 -> LayerNorm, with dummy matmuls holding the
                    # HAM clock gate at 8/8 through the tail
                    nc.scalar.activation(sqwarm[:], pairs[-1][:, 0:1], Sqrt,
                                         bias=epsc[:])
                    while pend:
                        pend.pop(0)()

                    def drain_s(s):
                        for c in range(KV - 4, KV):
                            u = c % 2
                            et2d = pairs[c // 2]
                            nc.tensor.matmul(
                                G[s][:],
                                lhsT=et2d[:, u * QW + s * P:
                                          u * QW + (s + 1) * P],
                                rhs=xE_r[:, c], start=False,
                                stop=(c == KV - 1))

                    drain_s(0)
                    gns3 = [None] * QS
                    for s in range(QS):
                        gns3[s] = slice_A(s)
                        if s + 1 < QS:
                            # next slice's drain fills the PE while DVE runs
                            # this slice's copy chain
                            drain_s(s + 1)
                        slice_BC(s, gns3[s])
                        ln_slice(s)

    nc.compile()
    return nc


def _prep_host(inputs, N, QW):
    """Host-side input resharding: transposes, folded weights, mask stripes."""
    import ml_dtypes
    x = np.ascontiguousarray(np.asarray(inputs["x"], dtype=np.float32))
    ei = np.asarray(inputs["edge_index"]).astype(np.int64)
    Wq = np.asarray(inputs["Wq"], dtype=np.float64)
    Wk = np.asarray(inputs["Wk"], dtype=np.float64)
    Wv = np.asarray(inputs["Wv"], dtype=np.float64)
    Wo = np.asarray(inputs["Wo"], dtype=np.float64)
    Wp = np.asarray(inputs["Wp"], dtype=np.float64)
    bq = np.asarray(inputs["bq"], dtype=np.float64)
    bk = np.asarray(inputs["bk"], dtype=np.float64)
    bv = np.asarray(inputs["bv"], dtype=np.float64)
    bo = np.asarray(inputs["bo"], dtype=np.float64)
    bp = np.asarray(inputs["bp"], dtype=np.float64)
    gamma = np.asarray(inputs["gamma"], dtype=np.float32)
    beta = np.asarray(inputs["beta"], dtype=np.float32)

    assert not bq.any() and not bk.any(), \
        "nonzero q/k biases not wired in the device graph"

    xT = np.ascontiguousarray(x.T)                       # [D, N]
    # folded score weight M_h = Wq_h^T Wk_h (x M x^T == q k^T)
    m_l = []
    for hh in range(H):
        m_l.append((Wq[hh].T @ Wk[hh]) * SCALE_QK)
    wqk_h = np.ascontiguousarray(np.stack(m_l).astype(np.float32))
    # folded v' weight and total bias
    wv_l, bias_tot = [], bp.copy()
    for hh in range(H):
        Wp_h = Wp[:, hh * D:(hh + 1) * D]                # [f, e']
        Gm = Wo[hh].T @ Wp_h.T                           # [e, f]
        wv_l.append(Wv[hh].T @ Gm)                       # [d, f]
        bias_tot = bias_tot + bo[hh] @ Wp_h.T + bv[hh] @ Gm
    wv_h = np.ascontiguousarray(np.stack(wv_l).astype(np.float32))

    gam_b = np.ascontiguousarray(np.broadcast_to(gamma, (P, D)).astype(np.float32))
    bet_b = np.ascontiguousarray(np.broadcast_to(beta, (P, D)).astype(np.float32))
    bia_b = np.ascontiguousarray(
        np.broadcast_to(bias_tot.astype(np.float32), (P, D)))

    KV = N // P
    D1 = D + 2
    # raw x chunks + ones columns, pre-arranged to the SBUF layout
    xE = np.ones((N, D1), dtype=np.float32)
    xE[:, :D] = x
    xE_b = np.ascontiguousarray(
        xE.reshape(KV, P, D1).transpose(1, 0, 2).reshape(P, KV * D1)
        .astype(ml_dtypes.bfloat16))

    # {0,1} mask stripes per core, pre-arranged to the SBUF layout
    # mall[p, c*QW + q] = adjacency[c*P + p, q0 + q]  (kv-major, symm+diag)
    adj = np.zeros((N, N), dtype=np.uint8)
    r, c = ei[0], ei[1]
    adj[r, c] = 1
    adj[c, r] = 1
    adj[np.arange(N), np.arange(N)] = 1
    malls = []
    for core in range(N_CORES):
        q0 = core * QW
        stripe = adj[:, q0:q0 + QW]                      # [N(kv), QW]
        m = stripe.reshape(KV, P, QW).transpose(1, 0, 2).reshape(P, KV * QW)
        malls.append(np.ascontiguousarray(m.astype(ml_dtypes.bfloat16)))
    return xT, wqk_h, wv_h, xE_b, gam_b, bet_b, bia_b, malls


_BUILD_CACHE = {}


def _run(inputs, trace=False, mask_dt_name="bfloat16", mode="bf16",
         tmpdir=None):
    from concourse.bass_utils import run_bass_kernel_spmd
    from concourse.bass_interp import get_hw_module
    import ml_dtypes

    N = int(np.asarray(inputs["x"]).shape[0])
    QW = N // N_CORES
    (xT, wqk_h, wv_h, xE_b, gam_b, bet_b, bia_b, malls) = \
        _prep_host(inputs, N, QW)

    bdt = ml_dtypes.bfloat16
    f8dt = ml_dtypes.float8_e4m3
    xT_8 = xT.astype(f8dt)
    wqk_8 = wqk_h.astype(f8dt)
    wv_b = wv_h.astype(bdt)
    idn_b = np.eye(P, dtype=np.float32).astype(bdt)

    gamma = np.asarray(inputs["gamma"], np.float64)
    beta = np.asarray(inputs["beta"], np.float64)
    key = (N, QW, mask_dt_name, mode, not np.any(bia_b),
           bool((gamma == 1).all()), not beta.any())
    nc = _BUILD_CACHE.get(key)
    if nc is None:
        nc = _build(N, QW, mask_dt_name=mask_dt_name, mode=mode,
                    triv_bias=key[4], triv_gamma=key[5], triv_beta=key[6])
        _BUILD_CACHE[key] = nc
    old = nc.m
    nc.m = get_hw_module(nc.m)
    try:
        in_maps = []
        for core in range(N_CORES):
            q0 = core * QW
            in_maps.append({
                "xT8": xT_8,
                "xq8": np.ascontiguousarray(xT_8[:, q0:q0 + QW]),
                "wqk8": wqk_8, "wv": wv_b, "xE": xE_b, "idn": idn_b,
                "gamma_b": gam_b, "beta_b": bet_b, "bias_b": bia_b,
                "mall": malls[core],
            })
        res = run_bass_kernel_spmd(nc, in_maps, core_ids=list(range(N_CORES)),
                                   trace=trace, tmpdir=tmpdir)
    finally:
        nc.m = old
    out = np.concatenate([res.results[i]["out"] for i in range(N_CORES)], axis=0)
    return out.astype(np.float32), res


def kernel(**inputs) -> np.ndarray:
    out, _ = _run(inputs)
    return out


# revision 68
# speedup vs baseline: 1.0630x; 1.0319x over previous
"""Trainium2 Bass kernel for nn_AdaptiveGraphConvLayer (graph multi-head attention).

Reference computation:
    mask = dense additive edge mask from edge_index (symmetric + self loops)
    per head h: q,k,v projections of x; scores = q @ k.T / 16 + mask; softmax
    o_h = attn @ v_h; head_out_h = o_h @ Wo_h.T + bo_h
    out = concat_h(head_out) @ Wp.T + bp;  LayerNorm(out) * gamma + beta
    (N=4096 nodes, D=256, H=4 heads, E=131072 edges; ~80 GFLOP)

Device strategy (node-parallel, zero collectives) — "factored" form:
  - Core c owns query rows [c*512, (c+1)*512) for ALL 4 heads.
  - Score-path fold:  scores = (x Wq^T)(x Wk^T)^T = x @ M @ x^T with
        M_h = Wq_h^T Wk_h   (host-precomputed, fp8 with a 64x scale)
    so the per-head K-projection over all 4096 nodes disappears: the
    scores matmul contracts raw fp8 x^T (landed once by DMA) against a
    tiny per-head tq = M^T x_q^T  [256, 512].
  - V-path fold:  o = attn^T (x Wv') = (attn^T x) Wv'  with
        Wv'_h = Wv_h^T (Wp_h Wo_h)^T   (host-precomputed)
    Stage 1 contracts the masked-exp tiles against RAW x chunks
    (head-independent, in SBUF once, ones-columns appended on host for
    the softmax denominators).  Stage 2 is a tiny per-head
    [512,256] @ [256,256] through PE transposes of the normalized G.
    The per-head V-projection over 4096 nodes and its PSUM->SBUF
    copies disappear.
  - fp8 DoubleRow matmuls for tq + scores; bf16 for stage 1/2.
  - Edge mask: host reshards edge_index into per-core dense {0,1} bf16
    stripes in SBUF layout; ONE DVE multiply per chunk right after its
    exp (the factored V path removed the projection copies that used
    to crowd DVE, so the mask is timely there; fp8 additive -240 mask
    via identity DR matmuls measured SLOWER: +27us PE LDWEIGHTS).
  - Stage-1 matmuls run TWO pairs behind the exp/mask pipeline so a
    queue hiccup ahead of an exp/mask in an engine FIFO eats lookahead,
    not PE time.  (Three pairs measured worse; pair-batched exps over
    [P,1024] PSUM tiles measured worse: 2 PSUM pair slots serialize
    scores(p+2) behind exp(p).)
  - fp32 PSUM accumulate, fp32 softmax-normalize/LayerNorm.
  - Head h's tail chain (normalize G, transpose, stage 2, acc) and head
    h+1's tq are emitted interleaved into head h+1's attention loop.
  - HAM clock gate: warmup matmuls hold 8/8 through the input-DMA window.
  - ACT activation-table sets: no set holds Exp+Sqrt.  The preamble
    loads the Exp set; Sqrt's set is loaded by a dummy activation
    anchored on the final et tile so the 1.3us switch hides under the
    last stage-1 matmuls instead of stalling the LN tail.
  - Tail: fused Square+accum_out variance, Sqrt table preloaded, affine
    LN ops elided when gamma/beta/bias are trivial for the given inputs.
"""

import numpy as np

N_FULL = 4096
D = 256
H = 4
N_CORES = 8
EPS = 1e-5
P = 128  # partitions

WARMUP = 3
SCALE_QK = 64.0  # host scale on M = Wq^T Wk so fp8 sees ~unit-rms values
# tq psum->fp8 copy scale: lands tq around unit rms (well clear of the
# fp8 denormal cliff); EXP_SCALE undoes SCALE_QK*TQ_BETA*sqrt(D)
TQ_BETA = 3.0 / 256.0
EXP_SCALE = 1.0 / 12.0


def _build(N, QW, mask_dt_name="bfloat16", mode="bf16",
           triv_bias=False, triv_gamma=False, triv_beta=False):
    """Build + compile the SPMD Bass graph (identical on all cores)."""
    import concourse.bacc as bacc
    import concourse.tile as tile
    import concourse.bass as bass
    from concourse import mybir

    f32 = mybir.dt.float32
    f8 = mybir.dt.float8e4
    mask_dt = getattr(mybir.dt, mask_dt_name)
    cdt = mybir.dt.bfloat16
    DR = mybir.MatmulPerfMode.DoubleRow
    Exp = mybir.ActivationFunctionType.Exp
    Sqrt = mybir.ActivationFunctionType.Sqrt
    AX = mybir.AxisListType.X
    MUL = mybir.AluOpType.mult
    KV = N // P            # kv chunks of 128
    NP2 = KV // 2          # kv chunk pairs
    QS = QW // P           # q slices of 128 within this core's window
    D1 = D + 2             # x + ones columns (even width)

    nc = bacc.Bacc("TRN2", target_bir_lowering=False, debug=False,
                   num_devices=N_CORES)

    xT8_d = nc.dram_tensor("xT8", [D, N], f8, kind="ExternalInput").ap()
    xq8_d = nc.dram_tensor("xq8", [D, QW], f8, kind="ExternalInput").ap()
    wqk8_d = nc.dram_tensor("wqk8", [H, D, D], f8, kind="ExternalInput").ap()
    xE_d = nc.dram_tensor("xE", [P, (N // P) * D1], cdt,
                          kind="ExternalInput").ap()
    wv_d = nc.dram_tensor("wv", [H, D, D], cdt, kind="ExternalInput").ap()
    idn_d = nc.dram_tensor("idn", [P, P], cdt, kind="ExternalInput").ap()
    gam_d = nc.dram_tensor("gamma_b", [P, D], f32, kind="ExternalInput").ap()
    bet_d = nc.dram_tensor("beta_b", [P, D], f32, kind="ExternalInput").ap()
    bia_d = nc.dram_tensor("bias_b", [P, D], f32, kind="ExternalInput").ap()
    mal_d = nc.dram_tensor("mall", [P, (N // P) * QW], mask_dt,
                           kind="ExternalInput").ap()
    out_d = nc.dram_tensor("out", [QW, D], cdt, kind="ExternalOutput").ap()

    with tile.TileContext(nc) as tc:
        with (
            tc.tile_pool(name="const", bufs=1) as cp,
            tc.tile_pool(name="tqp", bufs=2) as tqp,
            tc.tile_pool(name="maskp", bufs=1) as mp,
            tc.tile_pool(name="work", bufs=8) as wp,
            tc.tile_pool(name="accs", bufs=1) as ac,
            tc.tile_pool(name="ln", bufs=8) as lp,
            tc.tile_pool(name="psA", bufs=4, space="PSUM") as psA,
            tc.tile_pool(name="psG", bufs=1, space="PSUM") as psG,
        ):
            # ---------- PE warmup: dummy matmuls on uninitialized SBUF so
            # the HAM clock-gate reaches K=8/8 while input DMAs stream in.
            wu = cp.tile([P, 640], cdt, tag="wu")
            nc.vector.memset(wu[:], 0.125)
            wups = psA.tile([P, 512], f32, tag="ps", name="wups")
            for w in range(WARMUP):
                nc.tensor.matmul(wups[:], lhsT=wu[:, :P], rhs=wu[:, P:P + 512],
                                 start=True, stop=True)

            # ---------- load inputs into SBUF ----------
            # DMA queue is FIFO: land the tq inputs first so the first real
            # matmuls start as early as possible.
            xq8 = cp.tile([P, 2 * QW], f8, tag="xq8")
            nc.sync.dma_start(out=xq8[:].rearrange("p (i q) -> p i q", q=QW),
                              in_=xq8_d[:].rearrange("(i p) q -> p i q", p=P))
            wqk8 = cp.tile([P, H * 2 * D], f8, tag="wqk8")
            nc.sync.dma_start(
                out=wqk8[:].rearrange("p (h i d) -> p h i d", h=H, i=2),
                in_=wqk8_d[:].rearrange("h (i p) d -> p h i d", p=P))
            # xT8 / {0,1} mask / xE stripes land interleaved by quarter:
            # head-0 chunk c waits only for its own quarter, so quarter 0 of
            # all three goes before quarter 1 of any
            xT8 = cp.tile([P, 2 * N], f8, tag="xT8")
            Mall = mp.tile([P, KV * QW], mask_dt, tag="mask")
            xE = cp.tile([P, KV * D1], cdt, tag="xE")
            MQ = KV // 8
            for q8 in range(8):
                w = N // 8
                nc.sync.dma_start(
                    out=xT8[:].rearrange("p (i n) -> p i n", n=N)
                        [:, :, q8 * w:(q8 + 1) * w],
                    in_=xT8_d[:].rearrange("(i p) n -> p i n", p=P)
                        [:, :, q8 * w:(q8 + 1) * w])
                nc.sync.dma_start(
                    out=Mall[:, q8 * MQ * QW:(q8 + 1) * MQ * QW],
                    in_=mal_d[:, q8 * MQ * QW:(q8 + 1) * MQ * QW])
                nc.sync.dma_start(
                    out=xE[:, q8 * MQ * D1:(q8 + 1) * MQ * D1],
                    in_=xE_d[:, q8 * MQ * D1:(q8 + 1) * MQ * D1])
            wv = cp.tile([P, H * 2 * D], cdt, tag="wv")
            nc.sync.dma_start(
                out=wv[:].rearrange("p (h i d) -> p h i d", h=H, i=2),
                in_=wv_d[:].rearrange("h (i p) d -> p h i d", p=P))
            idn = cp.tile([P, P], cdt, tag="idn")
            nc.sync.dma_start(out=idn[:], in_=idn_d[:])
            gam = cp.tile([P, D], f32, tag="gam")
            bet = cp.tile([P, D], f32, tag="bet")
            bia = cp.tile([P, D], f32, tag="bia")
            if not triv_gamma:
                nc.sync.dma_start(out=gam[:], in_=gam_d[:])
            if not triv_beta:
                nc.sync.dma_start(out=bet[:], in_=bet_d[:])
            if not triv_bias:
                nc.sync.dma_start(out=bia[:], in_=bia_d[:])
            epsc = cp.tile([P, 1], f32, tag="epsc")
            nc.vector.memset(epsc[:], EPS)
            eps2 = cp.tile([P, 1], f32, tag="eps2")
            nc.vector.memset(eps2[:], float(D) * float(D) * EPS)
            # preload the Exp table set (covers Copy/Square too); Sqrt's set
            # is loaded late, anchored after the final exp
            sqwarm = cp.tile([P, 1], f32, tag="sqwarm")
            nc.scalar.activation(sqwarm[:], epsc[:], Exp, bias=epsc[:])

            xq8_r = xq8[:].rearrange("p (i q) -> p i q", i=2)
            xT8_r = xT8[:].rearrange("p (i n) -> p i n", i=2)
            wqk8_r = wqk8[:].rearrange("p (h i d) -> p h i d", h=H, i=2)
            xE_r = xE[:].rearrange("p (c e) -> p c e", e=D1)

            acc = [ac.tile([P, D], f32, tag=f"acc{s}", name=f"acc{s}")
                   for s in range(QS)]

            cpy = [0]

            def copy_eng():
                # 1:2 ACT:DVE (ACT carries the exp pipeline)
                cpy[0] += 1
                return nc.scalar if cpy[0] % 3 == 0 else nc.vector

            def emit_copy(dst, src):
                e = copy_eng()
                if e is nc.scalar:
                    e.copy(dst, src)
                else:
                    e.tensor_copy(dst, src)

            def mk_tq(h):
                """tq_h = (M_h^T x_q^T) as 2 fp8 DR planes [P, 2, QW]."""
                tq = tqp.tile([P, 2 * QW], f8, tag="tq", name=f"tq{h}")

                def emit():
                    Copy = mybir.ActivationFunctionType.Copy
                    for j in range(2):
                        ps = psA.tile([P, 512], f32, tag="ps",
                                      name=f"tq{h}ps{j}")
                        nc.tensor.matmul(
                            ps[:, :QW],
                            lhsT=wqk8_r[:, h, :, j * P:(j + 1) * P],
                            rhs=xq8_r, start=True, stop=True, perf_mode=DR)
                        # scaled copy: lands tq so scores psum is 12*s and
                        # the fp8 -240 mask add zeroes via exp
                        if j == 0:
                            nc.scalar.activation(tq[:, :QW], ps[:, :QW],
                                                 Copy, scale=TQ_BETA)
                        else:
                            nc.vector.tensor_scalar(
                                out=tq[:, QW:], in0=ps[:, :QW],
                                scalar1=TQ_BETA, scalar2=None, op0=MUL)
                return tq, emit

            inv_d = 1.0 / D
            Square = mybir.ActivationFunctionType.Square

            def ln_slice(s):
                """bias + LayerNorm + store for one q slice."""
                t = acc[s]
                if not triv_bias:
                    nc.vector.tensor_add(t[:], t[:], bia[:])
                # one-pass LN stats: sum on DVE and sum-of-squares on ACT in
                # parallel on t; then D^2*var = D*sumsq - sum^2 via [P,1]
                # ops.  LN is scale-invariant: y = (D*t - sum)/sqrt(D^2*var
                # + D^2*eps).
                musum = lp.tile([P, 1], f32, tag="musum")
                nc.vector.reduce_sum(musum[:], t[:], axis=AX)
                sq = lp.tile([P, D], f32, tag="sq")
                vs = lp.tile([P, 1], f32, tag="vs")
                nc.scalar.activation(sq[:], t[:], Square, accum_out=vs[:])
                xc = lp.tile([P, D], f32, tag="xc")
                nc.vector.tensor_scalar(out=xc[:], in0=t[:], scalar1=float(D),
                                        scalar2=musum[:],
                                        op0=MUL, op1=mybir.AluOpType.subtract)
                t1 = lp.tile([P, 1], f32, tag="t1")
                nc.vector.tensor_mul(t1[:], musum[:], musum[:])
                t2 = lp.tile([P, 1], f32, tag="t2")
                nc.vector.tensor_scalar(out=t2[:], in0=vs[:],
                                        scalar1=float(D), scalar2=t1[:],
                                        op0=MUL, op1=mybir.AluOpType.subtract)
                sd = lp.tile([P, 1], f32, tag="sd")
                nc.scalar.activation(sd[:], t2[:], Sqrt, bias=eps2[:])
                rs = lp.tile([P, 1], f32, tag="rs")
                nc.vector.reciprocal(rs[:], sd[:])
                og = lp.tile([P, D], cdt, tag="og")
                if triv_gamma:
                    nc.vector.tensor_scalar_mul(og[:], xc[:], rs[:])
                else:
                    nc.vector.scalar_tensor_tensor(og[:], in0=xc[:],
                                                   scalar=rs[:], in1=gam[:],
                                                   op0=MUL, op1=MUL)
                if triv_beta:
                    nc.sync.dma_start(out=out_d[s * P:(s + 1) * P, :],
                                      in_=og[:])
                else:
                    oo = lp.tile([P, D], cdt, tag="oo")
                    nc.vector.tensor_add(oo[:], og[:], bet[:])
                    nc.sync.dma_start(out=out_d[s * P:(s + 1) * P, :],
                                      in_=oo[:])

            # head 0's tq runs in the prologue
            tq0, emit0 = mk_tq(0)
            emit0()
            cur_tq = tq0

            pend = []  # thunks spread into the current head's kv loop

            for h in range(H):
                tq_r = cur_tq[:].rearrange("p (i q) -> p i q", i=2)
                G = [psG.tile([P, D1], f32, tag=f"G{s}", name=f"G{s}_{h}")
                     for s in range(QS)]

                def stage1(c, et2, G=G):
                    u = c % 2
                    for s in range(QS):
                        nc.tensor.matmul(
                            G[s][:],
                            lhsT=et2[:, u * QW + s * P:u * QW + (s + 1) * P],
                            rhs=xE_r[:, c], start=(c == 0), stop=(c == KV - 1))

                # next head's tq: tiny (2 DR matmuls + 2 copies); MUST be
                # emitted inside THIS head's loop so the next head's scores
                # reads are ordered after its writes
                if h + 1 < H:
                    nxt_tq, emit_tq = mk_tq(h + 1)
                else:
                    nxt_tq, emit_tq = None, None

                npend = len(pend)
                pairs = []
                et2 = None
                for c in range(KV):
                    u = c % 2
                    sc = psA.tile([P, 512], f32, tag="ps")
                    nc.tensor.matmul(sc[:, :QW],
                                     lhsT=xT8_r[:, :, c * P:c * P + P],
                                     rhs=tq_r, start=True, stop=True,
                                     perf_mode=DR)
                    if u == 0:
                        et2 = wp.tile([P, 2 * QW], cdt, tag="et")
                    nc.scalar.activation(et2[:, u * QW:(u + 1) * QW],
                                         sc[:, :QW], Exp, scale=EXP_SCALE)
                    # per-chunk {0,1} mask multiply on DVE right after the
                    # exp: the even half unblocks its stage-1 a chunk sooner
                    nc.vector.tensor_mul(et2[:, u * QW:(u + 1) * QW],
                                         et2[:, u * QW:(u + 1) * QW],
                                         Mall[:, c * QW:(c + 1) * QW])
                    if u == 1:
                        pairs.append(et2)
                    if c >= 4:
                        stage1(c - 4, pairs[(c - 4) // 2])
                    if c == 6 and emit_tq is not None:
                        emit_tq()
                    if c >= 2 and npend:
                        want = ((c - 1) * npend) // (KV - 2)
                        while npend - len(pend) < want and pend:
                            pend.pop(0)()

                def slice_A(s, G=G):
                    # reciprocal of the denominator column + UNNORMALIZED
                    # bf16 copy of G: the 1/denom folds into the acc update
                    # (keeps the PE transpose chain off the normalize)
                    rec = lp.tile([P, 1], f32, tag="rec")
                    nc.vector.reciprocal(rec[:], G[s][:, D:D + 1])
                    gn = lp.tile([P, D], cdt, tag="gn", name=f"gn{s}_{h}")
                    nc.vector.tensor_copy(gn[:], G[s][:, 0:D])
                    return (rec, gn)

                def slice_BC(s, recgn, h=h):
                    rec, gn = recgn
                    # bf16 transposes (psum bf16 tile = same 2KB footprint
                    # as the f32 scores tiles) + a 2x-mode gt copy
                    tp = psA.tile([P, 1024], cdt, tag="ps", name=f"tp{s}_{h}")
                    nc.tensor.transpose(tp[:, 0:P], gn[:, 0:P], idn[:])
                    nc.tensor.transpose(tp[:, P:2 * P], gn[:, P:2 * P],
                                        idn[:])
                    gt = lp.tile([P, 2 * P], cdt, tag="gt", name=f"gt{s}_{h}")
                    emit_copy(gt[:], tp[:, 0:2 * P])
                    s2 = psA.tile([P, 512], f32, tag="ps", name=f"s2{s}_{h}")
                    for i in range(2):
                        nc.tensor.matmul(
                            s2[:, :D], lhsT=gt[:, i * P:(i + 1) * P],
                            rhs=wv[:, (h * 2 + i) * D:(h * 2 + i + 1) * D],
                            start=(i == 0), stop=(i == 1))
                    if h == 0:
                        nc.vector.tensor_scalar_mul(acc[s][:], s2[:, :D],
                                                    rec[:])
                    else:
                        nc.vector.scalar_tensor_tensor(
                            acc[s][:], in0=s2[:, :D], scalar=rec[:],
                            in1=acc[s][:], op0=MUL,
                            op1=mybir.AluOpType.add)

                if h < H - 1:
                    for c in range(KV - 4, KV):
                        stage1(c, pairs[c // 2])
                    while pend:
                        pend.pop(0)()
                    # ---- head tail chain ----
                    # A: normalize G by its denominator, emitted NOW (the
                    # next head's stage-1 start=True on the same PSUM tags
                    # must be emitted after these reads); B/C spread into the
                    # next head's attention loop
                    gns = [slice_A(s) for s in range(QS)]

                    def mk_bc(s, gns=gns):
                        return lambda: slice_BC(s, gns[s])

                    for s in range(QS):
                        pend.append(mk_bc(s))
                    cur_tq = nxt_tq
                else:
                    # last head: per-slice pipelined drain -> normalize ->
                    # stage2 -> LayerNorm, with dummy matmuls holding the
                    # HAM clock gate at 8/8 through the tail
                    nc.scalar.activation(sqwarm[:], pairs[-1][:, 0:1], Sqrt,
                                         bias=epsc[:])
                    while pend:
                        pend.pop(0)()

                    def drain_s(s):
                        for c in range(KV - 4, KV):
                            u = c % 2
                            et2d = pairs[c // 2]
                            nc.tensor.matmul(
                                G[s][:],
                                lhsT=et2d[:, u * QW + s * P:
                                          u * QW + (s + 1) * P],
                                rhs=xE_r[:, c], start=False,
                                stop=(c == KV - 1))

                    drain_s(0)
                    gns3 = [None] * QS
                    for s in range(QS):
                        gns3[s] = slice_A(s)
                        if s + 1 < QS:
                            # next slice's drain fills the PE while DVE runs
                            # this slice's copy chain
                            drain_s(s + 1)
                        slice_BC(s, gns3[s])
                        ln_slice(s)

    nc.compile()
    return nc


def _prep_host(inputs, N, QW):
    """Host-side input resharding: transposes, folded weights, mask stripes."""
    import ml_dtypes
    x = np.ascontiguousarray(np.asarray(inputs["x"], dtype=np.float32))
    ei = np.asarray(inputs["edge_index"]).astype(np.int64)
    Wq = np.asarray(inputs["Wq"], dtype=np.float64)
    Wk = np.asarray(inputs["Wk"], dtype=np.float64)
    Wv = np.asarray(inputs["Wv"], dtype=np.float64)
    Wo = np.asarray(inputs["Wo"], dtype=np.float64)
    Wp = np.asarray(inputs["Wp"], dtype=np.float64)
    bq = np.asarray(inputs["bq"], dtype=np.float64)
    bk = np.asarray(inputs["bk"], dtype=np.float64)
    bv = np.asarray(inputs["bv"], dtype=np.float64)
    bo = np.asarray(inputs["bo"], dtype=np.float64)
    bp = np.asarray(inputs["bp"], dtype=np.float64)
    gamma = np.asarray(inputs["gamma"], dtype=np.float32)
    beta = np.asarray(inputs["beta"], dtype=np.float32)

    assert not bq.any() and not bk.any(), \
        "nonzero q/k biases not wired in the device graph"

    xT = np.ascontiguousarray(x.T)                       # [D, N]
    # folded score weight M_h = Wq_h^T Wk_h (x M x^T == q k^T)
    m_l = []
    for hh in range(H):
        m_l.append((Wq[hh].T @ Wk[hh]) * SCALE_QK)
    wqk_h = np.ascontiguousarray(np.stack(m_l).astype(np.float32))
    # folded v' weight and total bias
    wv_l, bias_tot = [], bp.copy()
    for hh in range(H):
        Wp_h = Wp[:, hh * D:(hh + 1) * D]                # [f, e']
        Gm = Wo[hh].T @ Wp_h.T                           # [e, f]
        wv_l.append(Wv[hh].T @ Gm)                       # [d, f]
        bias_tot = bias_tot + bo[hh] @ Wp_h.T + bv[hh] @ Gm
    wv_h = np.ascontiguousarray(np.stack(wv_l).astype(np.float32))

    gam_b = np.ascontiguousarray(np.broadcast_to(gamma, (P, D)).astype(np.float32))
    bet_b = np.ascontiguousarray(np.broadcast_to(beta, (P, D)).astype(np.float32))
    bia_b = np.ascontiguousarray(
        np.broadcast_to(bias_tot.astype(np.float32), (P, D)))

    KV = N // P
    D1 = D + 2
    # raw x chunks + ones columns, pre-arranged to the SBUF layout
    xE = np.ones((N, D1), dtype=np.float32)
    xE[:, :D] = x
    xE_b = np.ascontiguousarray(
        xE.reshape(KV, P, D1).transpose(1, 0, 2).reshape(P, KV * D1)
        .astype(ml_dtypes.bfloat16))

    # {0,1} mask stripes per core, pre-arranged to the SBUF layout
    # mall[p, c*QW + q] = adjacency[c*P + p, q0 + q]  (kv-major, symm+diag)
    adj = np.zeros((N, N), dtype=np.uint8)
    r, c = ei[0], ei[1]
    adj[r, c] = 1
    adj[c, r] = 1
    adj[np.arange(N), np.arange(N)] = 1
    malls = []
    for core in range(N_CORES):
        q0 = core * QW
        stripe = adj[:, q0:q0 + QW]                      # [N(kv), QW]
        m = stripe.reshape(KV, P, QW).transpose(1, 0, 2).reshape(P, KV * QW)
        malls.append(np.ascontiguousarray(m.astype(ml_dtypes.bfloat16)))
    return xT, wqk_h, wv_h, xE_b, gam_b, bet_b, bia_b, malls


_BUILD_CACHE = {}


def _run(inputs, trace=False, mask_dt_name="bfloat16", mode="bf16",
         tmpdir=None):
    from concourse.bass_utils import run_bass_kernel_spmd
    from concourse.bass_interp import get_hw_module
    import ml_dtypes

    N = int(np.asarray(inputs["x"]).shape[0])
    QW = N // N_CORES
    (xT, wqk_h, wv_h, xE_b, gam_b, bet_b, bia_b, malls) = \
        _prep_host(inputs, N, QW)

    bdt = ml_dtypes.bfloat16
    f8dt = ml_dtypes.float8_e4m3
    xT_8 = xT.astype(f8dt)
    wqk_8 = wqk_h.astype(f8dt)
    wv_b = wv_h.astype(bdt)
    idn_b = np.eye(P, dtype=np.float32).astype(bdt)

    gamma = np.asarray(inputs["gamma"], np.float64)
    beta = np.asarray(inputs["beta"], np.float64)
    key = (N, QW, mask_dt_name, mode, not np.any(bia_b),
           bool((gamma == 1).all()), not beta.any())
    nc = _BUILD_CACHE.get(key)
    if nc is None:
        nc = _build(N, QW, mask_dt_name=mask_dt_name, mode=mode,
                    triv_bias=key[4], triv_gamma=key[5], triv_beta=key[6])
        _BUILD_CACHE[key] = nc
    old = nc.m
    nc.m = get_hw_module(nc.m)
    try:
        in_maps = []
        for core in range(N_CORES):
            q0 = core * QW
            in_maps.append({
                "xT8": xT_8,
                "xq8": np.ascontiguousarray(xT_8[:, q0:q0 + QW]),
                "wqk8": wqk_8, "wv": wv_b, "xE": xE_b, "idn": idn_b,
                "gamma_b": gam_b, "beta_b": bet_b, "bias_b": bia_b,
                "mall": malls[core],
            })
        res = run_bass_kernel_spmd(nc, in_maps, core_ids=list(range(N_CORES)),
                                   trace=trace, tmpdir=tmpdir)
    finally:
        nc.m = old
    out = np.concatenate([res.results[i]["out"] for i in range(N_CORES)], axis=0)
    return out.astype(np.float32), res


def kernel(**inputs) -> np.ndarray:
    out, _ = _run(inputs)
    return out
